# revision 15
# baseline (speedup 1.0000x reference)
"""EulerAttention Trainium2 kernel (fp8-accelerated).

Per-core sharding: core c in 0..7 -> (batch b = c // 4, query block qb = c % 4,
1024 queries each).  Each core computes K/V (+ feature maps) for its whole
batch, Q features for its query block, then flash-style scores/softmax/AV.

Precision scheme:
- Q/K/V projections: 3-pass fp8 DoubleRow (xh*Wh + xl*Wh + xh*Wl with
  x = xh + xl, W = Wh + Wl fp8 splits; dropped xl*Wl ~ 2^-12 relative)
  giving ~16-bit effective mantissa at 0.75x the f32r matmul cost.
- Feature maps cos/sin(theta): turns-space range reduction + ACT Sin table
  (as the f32r baseline).  e-tiles 0-1 (features 1..256, the high-variance
  ones) are kept f32r; e-tiles 2-7 are mean-centered (analytic Gaussian
  center, fp8-rounded so the fold is exact) and stored fp8.
- Scores: e-tiles 0-1 f32r matmuls + e-tiles 2-7 single fp8 DoubleRow on the
  centered features.  The dropped center terms: q-side (dq . ck_bar) is
  row-constant (softmax-invariant, dropped); k-side (cq_bar . dk) is folded
  into the Exp via a per-key ACT bias computed by tiny fp8 matmuls (lnG);
  the const term drops.
- Softmax (no max-subtraction; logits bounded) + AV + rowsum: f32r.

kernel(**inputs) takes the full unsharded inputs from reference.setup_inputs()
and returns the full [B, S, D] output.
"""
import sys, math

sys.path.insert(0, "/opt/trn_rl_repo")

import numpy as np
import ml_dtypes

B, S, D = 2, 4096, 1024
NCORES = 8
QBLK = S // 4          # queries per core
ET = D // 128          # number of 128-row e/d tiles (8)
NF8 = ET - 2           # fp8 e-tiles (et2-7)
MAGIC = float(1.5 * 2**23)
TWOPI = 2.0 * math.pi
INV_SQRT_D = 1.0 / math.sqrt(D)
F8NP = ml_dtypes.float8_e4m3

_cache = {}


def _build_program(s_keys=S, s_q=QBLK, trace_sim=False):
    """Build the SPMD bass program. s_keys/s_q parameterizable for mini-tests."""
    import concourse.bass as bass
    from concourse import bacc
    import concourse.mybir as mybir
    import concourse.tile as tile
    from contextlib import ExitStack

    f32 = mybir.dt.float32
    f32r = mybir.dt.float32r
    f8 = mybir.dt.float8e4
    Act = mybir.ActivationFunctionType
    Alu = mybir.AluOpType
    DR = mybir.MatmulPerfMode.DoubleRow

    n_sblk = s_keys // 512       # key production blocks (4 t-tiles each)
    n_tt = s_keys // 128         # key tiles (t)
    n_tgrp = max(1, n_tt // 8)   # AV groups of 8 t-tiles
    tt_per_grp = n_tt // n_tgrp
    n_qsb = s_q // 512           # query production blocks
    NS = s_q                     # resident query width (free dim in phase 2)
    n_ns = NS // 512             # N-splits for matmuls over queries

    nc = bacc.Bacc("TRN2", target_bir_lowering=False, debug=False)

    xT = nc.dram_tensor("xT", [D, s_keys], f32, kind="ExternalInput").ap()
    xTq = nc.dram_tensor("xTq", [D, s_q], f32, kind="ExternalInput").ap()
    WH = {}
    WL = {}
    for w in ("q", "k", "v"):
        WH[w] = nc.dram_tensor(f"W{w}H", [D, D], f8, kind="ExternalInput").ap()
        WL[w] = nc.dram_tensor(f"W{w}L", [D, D], f8, kind="ExternalInput").ap()
    # f32r e-tile-0 weight columns (1/wavelength amplification needs >fp8 there)
    WQ0 = nc.dram_tensor("WQ0R", [D, 128], f32r, kind="ExternalInput").ap()
    WK0 = nc.dram_tensor("WK0R", [D, 128], f32r, kind="ExternalInput").ap()
    # packed per-partition constants, columns:
    #   sc2 x8 | bq2 x8 | bk2 x8 | bv x8 | -cqm_c x6 | -cqm_s x6 | -ckm_c x6 | -ckm_s x6
    CON = nc.dram_tensor("CON", [128, 4 * ET + 4 * NF8], f32, kind="ExternalInput").ap()
    # fp8 q-center columns for the lnG matmuls: cos et2-7 | sin et2-7
    CQM8 = nc.dram_tensor("CQM8", [128, 2 * NF8], f8, kind="ExternalInput").ap()

    OT = nc.dram_tensor("OT", [D, s_q], f32, kind="ExternalOutput").ap()

    with tile.TileContext(nc, trace_sim=trace_sim) as tc, ExitStack() as top:
        # ---- DRAM intermediates, split per block for fine-grained RAW deps ----
        dram = top.enter_context(tc.tile_pool(name="dram", bufs=1, space="DRAM"))
        CKa_d = [dram.tile([256, 512], f32r, tag=f"cka{i}", name=f"ckad{i}")
                 for i in range(n_sblk)]
        SKa_d = [dram.tile([256, 512], f32r, tag=f"ska{i}", name=f"skad{i}")
                 for i in range(n_sblk)]
        CKb_d = [dram.tile([NF8 * 128, 512], f8, tag=f"ckb{i}", name=f"ckbd{i}")
                 for i in range(n_sblk)]
        SKb_d = [dram.tile([NF8 * 128, 512], f8, tag=f"skb{i}", name=f"skbd{i}")
                 for i in range(n_sblk)]
        V_d = [dram.tile([512, D], f32r, tag=f"v{i}", name=f"vd{i}")
               for i in range(n_sblk)]

        # ---- constants (tiny, load first) ----
        cpool = top.enter_context(tc.tile_pool(name="consts", bufs=1))
        ctile = cpool.tile([128, 4 * ET + 4 * NF8], f32, tag="ctile")
        nc.sync.dma_start(ctile[:], CON[:])
        sc2 = [ctile[:, i : i + 1] for i in range(ET)]
        bq2 = [ctile[:, ET + i : ET + i + 1] for i in range(ET)]
        bk2 = [ctile[:, 2 * ET + i : 2 * ET + i + 1] for i in range(ET)]
        bvt = [ctile[:, 3 * ET + i : 3 * ET + i + 1] for i in range(ET)]
        ncq = [ctile[:, 4 * ET + i : 4 * ET + i + 1] for i in range(2 * NF8)]
        nck = [ctile[:, 4 * ET + 2 * NF8 + i : 4 * ET + 2 * NF8 + i + 1]
               for i in range(2 * NF8)]
        cqm8 = cpool.tile([128, 2 * NF8], f8, tag="cqm8")
        nc.sync.dma_start(cqm8[:], CQM8[:])
        ones_f = cpool.tile([128, 2], f32, tag="ones_f")
        nc.vector.memset(ones_f[:], 1.0)
        ones_col = cpool.tile([128, 2], f32r, tag="ones_col")  # [K=128, M=2] rowsum lhsT
        nc.vector.tensor_copy(ones_col[:], ones_f[:])
        ones_rf = cpool.tile([1, 128], f32, tag="ones_rf")
        nc.vector.memset(ones_rf[:], 1.0)
        ones_row = cpool.tile([1, 128], f32r, tag="ones_row")  # [K=1, M=128] bcast lhsT
        nc.vector.tensor_copy(ones_row[:], ones_rf[:])

        # ---- shared PSUM pool ----
        psum = top.enter_context(tc.tile_pool(name="psum", bufs=1, space="PSUM"))

        # ---- resident Q feature maps: et0-1 f32r, et2-7 centered fp8 ----
        qres = top.enter_context(tc.tile_pool(name="qres", bufs=1))
        cqr = qres.tile([128, 2 * NS], f32r, tag="cqr")
        sqr = qres.tile([128, 2 * NS], f32r, tag="sqr")
        cq8 = qres.tile([128, NF8 * NS], f8, tag="cq8")
        sq8 = qres.tile([128, NF8 * NS], f8, tag="sq8")

        # ---- fp8 weights: two slot sets; set A holds q then k, set B holds v ----
        wpool_ctx = tc.tile_pool(name="w", bufs=1)
        wpool = wpool_ctx.__enter__()
        wa_h = [wpool.tile([128, 2 * D], f8, tag=f"wah{j}", name=f"wah{j}") for j in range(4)]
        wa_l = [wpool.tile([128, 2 * D], f8, tag=f"wal{j}", name=f"wal{j}") for j in range(4)]
        wb_h = [wpool.tile([128, 2 * D], f8, tag=f"wbh{j}", name=f"wbh{j}") for j in range(4)]
        wb_l = [wpool.tile([128, 2 * D], f8, tag=f"wbl{j}", name=f"wbl{j}") for j in range(4)]
        w0r = [wpool.tile([128, 128], f32r, tag=f"w0r{d}", name=f"w0r{d}") for d in range(ET)]

        def load_w(tiles, src):
            for j in range(4):
                nc.sync.dma_start(
                    tiles[j][:].rearrange("p (two e) -> p two e", two=2),
                    src[2 * j * 128 : (2 * j + 2) * 128, :]
                    .rearrange("(two p) e -> p two e", p=128))

        def load_w0(src):
            for dd in range(ET):
                nc.sync.dma_start(w0r[dd][:], src[dd * 128 : (dd + 1) * 128, :])

        # ================= PHASE 1: projections + feature maps =================
        with tc.tile_pool(name="p1sb", bufs=2) as p1, \
             tc.tile_pool(name="p1chain", bufs=2) as pch:
            pps = psum

            def load_xblk(src_ap, col0):
                """x block [1024, 512] dram slice -> fp8 hi + fp8 lo + f32r."""
                src3 = src_ap[:, col0 : col0 + 512].rearrange("(d p) s -> p d s", p=128)
                b32 = p1.tile([128, ET * 512], f32, tag="xb32", name="xb32", bufs=1)
                nc.sync.dma_start(b32[:].rearrange("p (d s) -> p d s", d=ET), src3)
                xh = p1.tile([128, ET * 512], f8, tag="xh8", name="xh8")
                nc.gpsimd.dma_start(xh[:].rearrange("p (d s) -> p d s", d=ET), src3)
                xr = p1.tile([128, ET * 512], f32r, tag="xbr", name="xbr")
                nc.gpsimd.dma_start(xr[:].rearrange("p (d s) -> p d s", d=ET), src3)
                xl = p1.tile([128, ET * 512], f8, tag="xl8", name="xl8")
                nc.vector.tensor_tensor(xl[:], b32[:], xh[:], Alu.subtract)
                return xh, xl, xr

            def proj_psum(ps, xblk, w_h, w_l, et):
                """Proj into psum for one e-tile: et0 f32r (precision), else
                3-pass fp8 DR (xh*Wh + xl*Wh + xh*Wl), contraction over d."""
                xh, xl, xr = xblk
                if et == 0:
                    for dd in range(ET):
                        nc.tensor.matmul(ps, w0r[dd][:],
                                         xr[:, dd * 512 : (dd + 1) * 512],
                                         start=(dd == 0), stop=(dd == ET - 1))
                    return
                xh3 = xh[:].rearrange("p (d s) -> p d s", d=ET)
                xl3 = xl[:].rearrange("p (d s) -> p d s", d=ET)
                esl = slice(et * 128, (et + 1) * 128)
                first = True
                for xs, ws in ((xh3, w_h), (xl3, w_h), (xh3, w_l)):
                    for j in range(4):
                        lhsT = ws[j][:].rearrange("p (two e) -> p two e", two=2)[:, :, esl]
                        rhs = xs[:, 2 * j : 2 * j + 2, :]
                        last = (xs is xh3 and ws is w_l and j == 3)
                        nc.tensor.matmul(ps, lhsT, rhs,
                                         start=first, stop=last, perf_mode=DR)
                        first = False

            def feature_block(xblk, et, w_h, w_l, bias_tiles, ncen, c_dst, s_dst):
                """cos/sin feature tiles [128, 512] for one e-tile.
                et<2: c_dst/s_dst are f32r tiles written by ACT Sin directly.
                et>=2: centered via ACT Identity + bias, written as fp8."""
                ps = pps.tile([128, 512], f32, tag="proj", name="psf", bufs=2)
                proj_psum(ps[:], xblk, w_h, w_l, et)
                r = pch.tile([128, 512], f32, tag="r", name="r")
                nc.scalar.activation(r[:], ps[:], Act.Identity,
                                     scale=sc2[et][:], bias=bias_tiles[et][:])
                kk = pch.tile([128, 512], f32, tag="kk", name="kk")
                nc.vector.tensor_scalar(kk[:], r[:], MAGIC, MAGIC, Alu.add, Alu.subtract)
                f = pch.tile([128, 512], f32, tag="f", name="f")
                nc.vector.scalar_tensor_tensor(f[:], kk[:], -1.0, r[:],
                                               Alu.mult, Alu.add)
                if et < 2:
                    nc.scalar.activation(s_dst, f[:], Act.Sin, scale=TWOPI)
                else:
                    s32 = pch.tile([128, 512], f32, tag="s32", name="s32")
                    nc.scalar.activation(s32[:], f[:], Act.Sin, scale=TWOPI)
                    nc.scalar.activation(s_dst, s32[:], Act.Identity,
                                         bias=ncen[NF8 + et - 2][:])
                g = pch.tile([128, 512], f32, tag="kk", name="g")
                nc.vector.add_range_wrap(g[:], f[:], 0.25, 0.5, 1.0)
                if et < 2:
                    nc.scalar.activation(c_dst, g[:], Act.Sin, scale=TWOPI)
                else:
                    c32 = pch.tile([128, 512], f32, tag="s32", name="c32")
                    nc.scalar.activation(c32[:], g[:], Act.Sin, scale=TWOPI)
                    nc.scalar.activation(c_dst, c32[:], Act.Identity,
                                         bias=ncen[et - 2][:])

            # --- Q features ---
            load_w(wa_h, WH["q"])
            load_w(wa_l, WL["q"])
            load_w0(WQ0)
            xq_blocks = [load_xblk(xTq, 0)]
            if n_qsb > 1:
                xq_blocks.append(load_xblk(xTq, 512))
            for qsb in range(n_qsb):
                xqb = xq_blocks[qsb]
                for et in range(ET):
                    if et < 2:
                        cd = cqr[:, et * NS + qsb * 512 : et * NS + qsb * 512 + 512]
                        sd = sqr[:, et * NS + qsb * 512 : et * NS + qsb * 512 + 512]
                    else:
                        e8 = et - 2
                        cd = cq8[:, e8 * NS + qsb * 512 : e8 * NS + qsb * 512 + 512]
                        sd = sq8[:, e8 * NS + qsb * 512 : e8 * NS + qsb * 512 + 512]
                    feature_block(xqb, et, wa_h, wa_l, bq2, ncq, cd, sd)

            # v weights into set B; k weights overwrite set A (WAR via Tile)
            load_w(wb_h, WH["v"])
            load_w(wb_l, WL["v"])
            load_w(wa_h, WH["k"])
            load_w(wa_l, WL["k"])
            load_w0(WK0)

            # --- K features + V ---
            for sblk in range(n_sblk):
                xkb = load_xblk(xT, sblk * 512)
                for et in range(ET):
                    if et < 2:
                        cst = pch.tile([128, 512], f32r, tag="cst", name="cst")
                        sst = pch.tile([128, 512], f32r, tag="sst", name="sst")
                        feature_block(xkb, et, wa_h, wa_l, bk2, nck, cst[:], sst[:])
                        nc.sync.dma_start(
                            CKa_d[sblk][et * 128 : (et + 1) * 128, :], cst[:])
                        nc.sync.dma_start(
                            SKa_d[sblk][et * 128 : (et + 1) * 128, :], sst[:])
                    else:
                        e8 = et - 2
                        cst = pch.tile([128, 512], f8, tag="cst8", name="cst8")
                        sst = pch.tile([128, 512], f8, tag="sst8", name="sst8")
                        feature_block(xkb, et, wa_h, wa_l, bk2, nck, cst[:], sst[:])
                        nc.sync.dma_start(
                            CKb_d[sblk][e8 * 128 : (e8 + 1) * 128, :], cst[:])
                        nc.sync.dma_start(
                            SKb_d[sblk][e8 * 128 : (e8 + 1) * 128, :], sst[:])
                # V in natural [t, dv] layout, no bias (folded into output)
                xh, xl, _xr = xkb
                xh3 = xh[:].rearrange("p (d s) -> p d s", d=ET)
                xl3 = xl[:].rearrange("p (d s) -> p d s", d=ET)
                for ti in range(4):
                    tsl = slice(ti * 128, (ti + 1) * 128)
                    for dg in range(2):
                        psv = pps.tile([128, 512], f32, tag="proj", name="psv", bufs=2)
                        first = True
                        for xs, ws in ((xh3, wb_h), (xl3, wb_h), (xh3, wb_l)):
                            for j in range(4):
                                lhsT = xs[:, 2 * j : 2 * j + 2, tsl]
                                rhs = (ws[j][:].rearrange("p (two e) -> p two e", two=2)
                                       [:, :, dg * 512 : dg * 512 + 512])
                                last = (xs is xh3 and ws is wb_l and j == 3)
                                nc.tensor.matmul(psv[:], lhsT, rhs,
                                                 start=first, stop=last, perf_mode=DR)
                                first = False
                        vstg = p1.tile([128, 512], f32r, tag="vstg", name="vstg")
                        # descale the x64 weight scaling
                        nc.vector.tensor_scalar(vstg[:], psv[:], 1.0 / 64.0, None,
                                                Alu.mult)
                        nc.sync.dma_start(
                            V_d[sblk][ti * 128 : (ti + 1) * 128,
                                      dg * 512 : (dg + 1) * 512], vstg[:])

        wpool_ctx.__exit__(None, None, None)

        # ================= PHASE 2: scores + softmax + AV =================
        with tc.tile_pool(name="p2sb", bufs=2) as p2, \
             tc.tile_pool(name="epool", bufs=tt_per_grp + 1) as epool, \
             tc.tile_pool(name="vpool", bufs=8) as vpool, \
             tc.tile_pool(name="oacc", bufs=1) as oacc:
            p2ps = psum

            o_ac = [oacc.tile([128, NS], f32, tag=f"o{dt}", name=f"oac{dt}")
                    for dt in range(ET)]
            ps_rs = p2ps.tile([2, NS], f32, tag="rs", bufs=1)

            for tg in range(n_tgrp):
                e_tiles = []
                for ti in range(tt_per_grp):
                    tt = tg * tt_per_grp + ti
                    sb_i, loc = tt // 4, tt % 4
                    cka = p2.tile([128, 2 * 128], f32r, tag="cka", name="cka")
                    ska = p2.tile([128, 2 * 128], f32r, tag="ska", name="ska")
                    ckb = p2.tile([128, NF8 * 128], f8, tag="ckb", name="ckb")
                    skb = p2.tile([128, NF8 * 128], f8, tag="skb", name="skb")
                    for dst, src, ne in ((cka, CKa_d[sb_i], 2), (ska, SKa_d[sb_i], 2),
                                         (ckb, CKb_d[sb_i], NF8), (skb, SKb_d[sb_i], NF8)):
                        nc.sync.dma_start(
                            dst[:].rearrange("p (e t) -> p e t", e=ne),
                            src[:, loc * 128 : (loc + 1) * 128]
                            .rearrange("(e p) t -> p e t", p=128))
                    ckb3 = ckb[:].rearrange("p (e t) -> p e t", e=NF8)
                    skb3 = skb[:].rearrange("p (e t) -> p e t", e=NF8)
                    # lnG: per-key fold of the q-center term (tiny fp8 matmuls)
                    psg = p2ps.tile([128, 2], f32, tag="proj", name="psg", bufs=2)
                    for e6 in range(NF8):
                        nc.tensor.matmul(psg[:, :1],
                                         ckb[:, e6 * 128 : (e6 + 1) * 128],
                                         cqm8[:, e6 : e6 + 1],
                                         start=(e6 == 0), stop=False)
                    for e6 in range(NF8):
                        nc.tensor.matmul(psg[:, :1],
                                         skb[:, e6 * 128 : (e6 + 1) * 128],
                                         cqm8[:, NF8 + e6 : NF8 + e6 + 1],
                                         start=False, stop=(e6 == NF8 - 1))
                    gb = p2.tile([128, 1], f32, tag="gb", name="gb")
                    nc.scalar.activation(gb[:], psg[:, :1], Act.Identity,
                                         scale=INV_SQRT_D)
                    ps_sim = p2ps.tile([128, NS], f32, tag="big", name="ps_sim", bufs=2)
                    cq83 = cq8[:].rearrange("p (e n) -> p e n", e=NF8)
                    sq83 = sq8[:].rearrange("p (e n) -> p e n", e=NF8)
                    for ns in range(n_ns):
                        sl = slice(ns * 512, ns * 512 + 512)
                        for et in range(2):
                            nc.tensor.matmul(ps_sim[:, sl],
                                             cka[:, et * 128 : (et + 1) * 128],
                                             cqr[:, et * NS + ns * 512 : et * NS + ns * 512 + 512],
                                             start=(et == 0), stop=False)
                            nc.tensor.matmul(ps_sim[:, sl],
                                             ska[:, et * 128 : (et + 1) * 128],
                                             sqr[:, et * NS + ns * 512 : et * NS + ns * 512 + 512],
                                             start=False, stop=False)
                        for pr in range(NF8 // 2):
                            nc.tensor.matmul(ps_sim[:, sl],
                                             ckb3[:, 2 * pr : 2 * pr + 2, :],
                                             cq83[:, 2 * pr : 2 * pr + 2, sl],
                                             start=False, stop=False, perf_mode=DR)
                            nc.tensor.matmul(ps_sim[:, sl],
                                             skb3[:, 2 * pr : 2 * pr + 2, :],
                                             sq83[:, 2 * pr : 2 * pr + 2, sl],
                                             start=False,
                                             stop=(pr == NF8 // 2 - 1), perf_mode=DR)
                    et_t = epool.tile([128, NS], f32r, tag="e", name="e")
                    nc.scalar.activation(et_t[:], ps_sim[:], Act.Exp,
                                         scale=INV_SQRT_D, bias=gb[:])
                    e_tiles.append((tt, et_t))
                    for ns in range(n_ns):
                        sl = slice(ns * 512, ns * 512 + 512)
                        nc.tensor.matmul(ps_rs[:, sl], ones_col[:], et_t[:, sl],
                                         start=(tt == 0), stop=(tt == n_tt - 1))
                # AV for this group
                for dg in range(2):
                    vts = []
                    for gi, (tt, _) in enumerate(e_tiles):
                        sb_i, loc = tt // 4, tt % 4
                        vt = vpool.tile([128, 512], f32r, tag="vt", name="vt")
                        nc.sync.dma_start(
                            vt[:], V_d[sb_i][loc * 128 : (loc + 1) * 128,
                                             dg * 512 : (dg + 1) * 512])
                        vts.append(vt)
                    for di in range(4):
                        dt = dg * 4 + di
                        ps_o = p2ps.tile([128, NS], f32, tag="big", name="ps_o", bufs=2)
                        for gi, (tt, et_t) in enumerate(e_tiles):
                            for ns in range(n_ns):
                                sl = slice(ns * 512, ns * 512 + 512)
                                nc.tensor.matmul(
                                    ps_o[:, sl], vts[gi][:, di * 128 : (di + 1) * 128],
                                    et_t[:, sl],
                                    start=(gi == 0), stop=(gi == len(e_tiles) - 1))
                        if tg == 0:
                            nc.vector.tensor_copy(o_ac[dt][:], ps_o[:])
                        else:
                            nc.vector.tensor_tensor(o_ac[dt][:], ps_o[:], o_ac[dt][:],
                                                    Alu.add)

            # normalize: recip of rowsum, broadcast via rank-1 matmul; + V bias
            rs_sb = p2.tile([1, NS], f32, tag="rs_sb")
            nc.vector.tensor_copy(rs_sb[:], ps_rs[:1, :])
            rec_f = p2.tile([1, NS], f32, tag="rec_f")
            nc.vector.reciprocal(rec_f[:], rs_sb[:])
            rec = p2.tile([1, NS], f32r, tag="rec")
            nc.vector.tensor_copy(rec[:], rec_f[:])
            ps_bc = p2ps.tile([128, NS], f32, tag="big", name="ps_bc", bufs=2)
            for ns in range(n_ns):
                sl = slice(ns * 512, ns * 512 + 512)
                nc.tensor.matmul(ps_bc[:, sl], ones_row[:], rec[:, sl],
                                 start=True, stop=True)
            bc = p2.tile([128, NS], f32, tag="bc")
            nc.vector.tensor_copy(bc[:], ps_bc[:])
            for dt in range(ET):
                on = p2.tile([128, NS], f32, tag="on", name="on")
                nc.vector.tensor_tensor(on[:], o_ac[dt][:], bc[:], Alu.mult)
                # per-partition V-bias add on ACT (idle at the tail)
                nc.scalar.activation(on[:], on[:], Act.Identity, bias=bvt[dt][:])
                nc.sync.dma_start(OT[dt * 128 : (dt + 1) * 128, :], on[:])

    nc.compile()
    return nc


def _f8(a):
    return np.asarray(a, np.float32).astype(F8NP)


def _host_prep(x, Wq, bq, Wk, bk, Wv, bv, phase_bias):
    wavelengths = np.arange(1, D + 1, dtype=np.float32) * np.float32(2.0 * math.pi / D)
    inv_wl = (np.float32(1.0) / (wavelengths + np.float32(1e-8))).astype(np.float32)
    sc2 = (inv_wl / TWOPI).astype(np.float32).reshape(ET, 128).copy()
    sc2[1:] /= np.float32(64.0)   # descale the x64 fp8 weight scaling (et1-7)
    bq2 = ((bq * inv_wl + phase_bias) / TWOPI).astype(np.float32).reshape(ET, 128)
    bk2 = ((bk * inv_wl + phase_bias) / TWOPI).astype(np.float32).reshape(ET, 128)

    # fp8 hi/lo weight splits, scaled x64 to clear the e4m3 subnormal floor
    W8 = {}
    W0R = {}
    for nm, W in (("q", Wq), ("k", Wk), ("v", Wv)):
        WT = np.ascontiguousarray(W.T).astype(np.float32) * np.float32(64.0)
        wh = _f8(WT)
        wl = _f8(WT - wh.astype(np.float32))
        W8[nm] = (wh, wl)
        if nm != "v":
            W0R[nm] = np.ascontiguousarray(W.T[:, :128]).astype(np.float32)

    # analytic feature centers (Gaussian approx), fp8-rounded so folds are exact
    def centers(W, bias):
        mu = (bias * inv_wl + phase_bias).astype(np.float64)
        sg = np.sqrt((W.astype(np.float64) ** 2).sum(1)) * inv_wl
        att = np.exp(-(sg ** 2) / 2)
        cc = (np.cos(mu) * att).astype(np.float32)
        ss = (np.sin(mu) * att).astype(np.float32)
        return _f8(cc).astype(np.float32), _f8(ss).astype(np.float32)

    cqm_c, cqm_s = centers(Wq, bq)   # [D]
    ckm_c, ckm_s = centers(Wk, bk)

    # CON columns: sc2 | bq2 | bk2 | bv | -cqm(c,s) et2-7 | -ckm(c,s) et2-7
    cols = [sc2, bq2, bk2, bv.reshape(ET, 128).astype(np.float32),
            (-cqm_c.reshape(ET, 128)[2:]), (-cqm_s.reshape(ET, 128)[2:]),
            (-ckm_c.reshape(ET, 128)[2:]), (-ckm_s.reshape(ET, 128)[2:])]
    con = np.concatenate(cols, axis=0)           # [(4*ET + 4*NF8), 128]
    con = np.ascontiguousarray(con.T).astype(np.float32)

    cqm8 = np.concatenate([cqm_c.reshape(ET, 128)[2:], cqm_s.reshape(ET, 128)[2:]],
                          axis=0)                # [2*NF8, 128]
    cqm8 = np.ascontiguousarray(cqm8.T)          # [128, 2*NF8]
    cqm8 = _f8(cqm8)

    xT = [np.ascontiguousarray(x[b].T).astype(np.float32) for b in range(x.shape[0])]
    return xT, W8, W0R, con, cqm8


def kernel(x, Wq, bq, Wk, bk, Wv, bv, phase_bias, _trace=False):
    from concourse.bass_utils import run_bass_kernel_spmd

    x = np.asarray(x, dtype=np.float32)
    xT, W8, W0R, con, cqm8 = _host_prep(
        x, np.asarray(Wq, np.float32), np.asarray(bq, np.float32),
        np.asarray(Wk, np.float32), np.asarray(bk, np.float32),
        np.asarray(Wv, np.float32), np.asarray(bv, np.float32),
        np.asarray(phase_bias, np.float32))

    if "prog" not in _cache:
        _cache["prog"] = _build_program()
    nc = _cache["prog"]

    in_maps = []
    for c in range(NCORES):
        b, qb = c // 4, c % 4
        in_maps.append({
            "xT": xT[b],
            "xTq": np.ascontiguousarray(xT[b][:, qb * QBLK : (qb + 1) * QBLK]),
            "WqH": W8["q"][0], "WqL": W8["q"][1],
            "WkH": W8["k"][0], "WkL": W8["k"][1],
            "WvH": W8["v"][0], "WvL": W8["v"][1],
            "WQ0R": W0R["q"], "WK0R": W0R["k"],
            "CON": con, "CQM8": cqm8,
        })
    res = run_bass_kernel_spmd(nc, in_maps, core_ids=list(range(NCORES)),
                               trace=_trace)
    out = np.empty((B, S, D), dtype=np.float32)
    for c in range(NCORES):
        b, qb = c // 4, c % 4
        out[b, qb * QBLK : (qb + 1) * QBLK, :] = res.results[c]["OT"].T
    if _trace:
        kernel.last_exec_time_ns = res.exec_time_ns
        kernel.last_result = res
    return out
